# Initial kernel scaffold
#
"""Segment-normalize kernel for trn2, 8 NeuronCores, batch-parallel.

Layout trick: host transposes x to [B, F, S] so features sit on SBUF
partitions and positions on the free dim. Segment stats then reduce along
the free dim (bn_stats per segment chunk), and the normalize is a fused
per-segment x*A + C with per-partition scalars (A = w*rstd,
C = b - mean*w*rstd).

The device program is specialized at trace time on the segment boundary
list (derived from change_points on the host); compiled NEFFs are cached
per boundary signature.
"""

import numpy as np

import concourse.bass as bass
from concourse import mybir
from concourse.bass_utils import run_bass_kernel_spmd

B, S, F = 16, 8192, 256
NCORES = 8
BPC = B // NCORES           # batches per core
NUNITS = BPC * 2            # (batch, feature-half) units per core, F=256 -> 2 halves
EPS = 1e-5
CHUNK = 512                 # bn_stats hardware max free size
VE_NORM_FRAC = 0.55         # fraction of positions normalized on VectorE (rest on ScalarE)

_cache: dict = {}


def _plan(change_points: np.ndarray):
    ind = (change_points.sum(axis=0) > 0)
    ind[0] = False
    bpos = np.flatnonzero(ind)
    starts = np.concatenate([[0], bpos]).astype(np.int64)
    ends = np.concatenate([bpos, [S]]).astype(np.int64)
    segs = [(int(s), int(e - s)) for s, e in zip(starts, ends)]
    nseg = len(segs)
    # columns: col k (k < nseg) = first chunk of segment k; extra chunks appended
    cols = []          # (start, clen)
    extras = []        # (owner_seg, col_idx) for extra chunks
    for k, (s0, ln) in enumerate(segs):
        cols.append((s0, min(ln, CHUNK)))
    for k, (s0, ln) in enumerate(segs):
        off = CHUNK
        while off < ln:
            cl = min(CHUNK, ln - off)
            extras.append((k, len(cols)))
            cols.append((s0 + off, cl))
            off += CHUNK
    ncol = len(cols)
    ncolp = (ncol + 3) // 4 * 4
    nsegp = (nseg + 3) // 4 * 4
    ce = np.zeros(ncolp, np.float32)
    co = np.zeros(ncolp, np.float32)
    for j, (s0, cl) in enumerate(cols):
        ce[j] = (cl + 1) // 2
        co[j] = cl // 2
    invl = np.zeros(nsegp, np.float32)
    for k, (s0, ln) in enumerate(segs):
        invl[k] = 1.0 / ln
    return segs, cols, extras, nseg, ncol, nsegp, ncolp, ce, co, invl


def _build(segs, cols, extras, nseg, ncol, nsegp, ncolp):
    f32 = mybir.dt.float32
    nc = bass.Bass()
    xt = nc.declare_dram_parameter("xt", [BPC, F, S], f32, isOutput=False)
    cearr = nc.declare_dram_parameter("ce", [ncolp], f32, isOutput=False)
    coarr = nc.declare_dram_parameter("co", [ncolp], f32, isOutput=False)
    invlarr = nc.declare_dram_parameter("invl", [nsegp], f32, isOutput=False)
    wb = nc.declare_dram_parameter("wb", [128, 6], f32, isOutput=False)
    yt = nc.declare_dram_parameter("yt", [BPC, F, S], f32, isOutput=True)

    # position split for normalize work between ACT and DVE: VE takes the
    # trailing segments covering ~VE_NORM_FRAC of positions
    split = nseg
    acc = 0
    for k in range(nseg - 1, -1, -1):
        if acc + segs[k][1] > VE_NORM_FRAC * S:
            break
        acc += segs[k][1]
        split = k
    n_const = 6  # ce, co, invl, wb rows loaded in 6 dmas (wb in 3? -> see below)

    from contextlib import ExitStack
    ctx = ExitStack()
    with ctx:
        xa = ctx.enter_context(nc.sbuf_tensor([128, S], f32))
        xb = ctx.enter_context(nc.sbuf_tensor([128, S], f32))
        xc = ctx.enter_context(nc.sbuf_tensor([128, S], f32))
        ya = ctx.enter_context(nc.sbuf_tensor([128, S], f32))
        yb = ctx.enter_context(nc.sbuf_tensor([128, S], f32))
        s6a = ctx.enter_context(nc.sbuf_tensor([128, ncolp, 6], f32))
        s6b = ctx.enter_context(nc.sbuf_tensor([128, ncolp, 6], f32))
        t1 = ctx.enter_context(nc.sbuf_tensor([128, ncolp], f32))
        t2 = ctx.enter_context(nc.sbuf_tensor([128, ncolp], f32))
        tsum = ctx.enter_context(nc.sbuf_tensor([128, ncolp], f32))
        tssq = ctx.enter_context(nc.sbuf_tensor([128, ncolp], f32))
        tmean = ctx.enter_context(nc.sbuf_tensor([128, nsegp], f32))
        tvar = ctx.enter_context(nc.sbuf_tensor([128, nsegp], f32))
        tstd = ctx.enter_context(nc.sbuf_tensor([128, nsegp], f32))
        trstd = ctx.enter_context(nc.sbuf_tensor([128, nsegp], f32))
        Aa = ctx.enter_context(nc.sbuf_tensor([128, nsegp], f32))
        Ab = ctx.enter_context(nc.sbuf_tensor([128, nsegp], f32))
        Ca = ctx.enter_context(nc.sbuf_tensor([128, nsegp], f32))
        Cb = ctx.enter_context(nc.sbuf_tensor([128, nsegp], f32))
        ce_t = ctx.enter_context(nc.sbuf_tensor([128, ncolp], f32))
        co_t = ctx.enter_context(nc.sbuf_tensor([128, ncolp], f32))
        invl_t = ctx.enter_context(nc.sbuf_tensor([128, nsegp], f32))
        w_t = ctx.enter_context(nc.sbuf_tensor([128, 2], f32))
        nw_t = ctx.enter_context(nc.sbuf_tensor([128, 2], f32))
        b_t = ctx.enter_context(nc.sbuf_tensor([128, 2], f32))
        eps_t = ctx.enter_context(nc.sbuf_tensor([128, 1], f32))
        LDC = ctx.enter_context(nc.semaphore("LDC"))
        LD = ctx.enter_context(nc.semaphore("LD"))
        VE1 = ctx.enter_context(nc.semaphore("VE1"))
        AC1 = ctx.enter_context(nc.semaphore("AC1"))
        VEs = ctx.enter_context(nc.semaphore("VEs"))
        ACs = ctx.enter_context(nc.semaphore("ACs"))
        VEN = ctx.enter_context(nc.semaphore("VEN"))
        ST = ctx.enter_context(nc.semaphore("ST"))
        block = ctx.enter_context(nc.Block())
        xtile = [xa, xb, xc]
        ytile = [ya, yb]
        s6 = [s6a, s6b]
        At = [Aa, Ab]
        Ct = [Ca, Cb]

        def unit_dram(u):
            bi, fh = u // 2, u % 2
            return (xt[bi, fh * 128:(fh + 1) * 128, :],
                    yt[bi, fh * 128:(fh + 1) * 128, :])

        @block.gpsimd
        def _(g):
            def bcast(dram, n):
                ap = dram[:]
                return bass.AP(tensor=ap.tensor, offset=ap.offset,
                               ap=[[0, 128], [1, n]])
            g.dma_start(out=ce_t[:, :], in_=bcast(cearr, ncolp)).then_inc(LDC, 16)
            g.dma_start(out=co_t[:, :], in_=bcast(coarr, ncolp)).then_inc(LDC, 16)
            g.dma_start(out=invl_t[:, :], in_=bcast(invlarr, nsegp)).then_inc(LDC, 16)
            g.dma_start(out=w_t[:, :], in_=wb[:, 0:2]).then_inc(LDC, 16)
            g.dma_start(out=nw_t[:, :], in_=wb[:, 2:4]).then_inc(LDC, 16)
            g.dma_start(out=b_t[:, :], in_=wb[:, 4:6]).then_inc(LDC, 16)

        @block.sync
        def _(sp):
            # 3 loads up-front (triple-buffered x); per unit: load u+3 (x slot
            # free once unit u fully read), then store u
            for u0 in range(min(3, NUNITS)):
                xd, _ = unit_dram(u0)
                sp.dma_start(out=xtile[u0 % 3][:, :], in_=xd).then_inc(LD, 16)
            for u in range(NUNITS):
                sp.wait_ge(ACs, u + 1)
                sp.wait_ge(VEN, u + 1)
                if u + 3 < NUNITS:
                    xd, _ = unit_dram(u + 3)
                    sp.dma_start(out=xtile[u % 3][:, :], in_=xd).then_inc(LD, 16)
                _, yd = unit_dram(u)
                sp.dma_start(out=yd, in_=ytile[u % 2][:, :]).then_inc(ST, 16)

        @block.vector
        def _(ve):
            nc.vector.memset(eps_t[:, :], EPS)
            nc.vector.memset(s6a[:, :, :], 0.0)
            nc.vector.memset(s6b[:, :, :], 0.0)
            ve.wait_ge(LDC, 16 * n_const)

            def ve_share(v):
                if v >= 2:
                    ve.wait_ge(ST, 16 * (v - 1))  # y buffer reuse
                xv = xtile[v % 3]
                for k in range(split, nseg):
                    s0, ln = segs[k]
                    nc.vector.tensor_scalar(
                        out=ytile[v % 2][:, s0:s0 + ln], in0=xv[:, s0:s0 + ln],
                        scalar1=At[v % 2][:, k:k + 1], scalar2=Ct[v % 2][:, k:k + 1],
                        op0=mybir.AluOpType.mult, op1=mybir.AluOpType.add)
                nc.vector.memset(t2[:, 0:1], 0.0).then_inc(VEN, 1)

            for u in range(NUNITS):
                fh = u % 2
                xu = xtile[u % 3]
                ve.wait_ge(LD, 16 * (u + 1))
                if u >= 2:
                    ve.wait_ge(ACs, u - 1)  # A/C buffer reuse
                s = s6[u % 2]
                for j, (s0, cl) in enumerate(cols):
                    nc.vector.bn_stats(out=s[:, j, :], in_=xu[:, s0:s0 + cl])
                m_e = s[:, :, 1]
                s_e = s[:, :, 2]
                m_o = s[:, :, 4]
                s_o = s[:, :, 5]
                nc.vector.tensor_mul(out=t1[:, :], in0=m_e, in1=ce_t[:, :])
                nc.vector.tensor_mul(out=t2[:, :], in0=m_o, in1=co_t[:, :])
                nc.vector.tensor_add(out=tsum[:, :], in0=t1[:, :], in1=t2[:, :])
                nc.vector.tensor_add(out=tssq[:, :], in0=s_e, in1=s_o)
                nc.vector.tensor_mul(out=t1[:, :], in0=m_e, in1=t1[:, :])
                nc.vector.tensor_add(out=tssq[:, :], in0=tssq[:, :], in1=t1[:, :])
                nc.vector.tensor_mul(out=t2[:, :], in0=m_o, in1=t2[:, :])
                nc.vector.tensor_add(out=tssq[:, :], in0=tssq[:, :], in1=t2[:, :])
                for k, j in extras:
                    nc.vector.tensor_add(out=tsum[:, k:k + 1], in0=tsum[:, k:k + 1], in1=tsum[:, j:j + 1])
                    nc.vector.tensor_add(out=tssq[:, k:k + 1], in0=tssq[:, k:k + 1], in1=tssq[:, j:j + 1])
                nc.vector.tensor_mul(out=tmean[:, :], in0=tsum[:, 0:nsegp], in1=invl_t[:, :])
                nc.vector.tensor_mul(out=tvar[:, :], in0=tssq[:, 0:nsegp], in1=invl_t[:, :])
                nc.vector.tensor_mul(out=t1[:, 0:nsegp], in0=tmean[:, :], in1=tmean[:, :])
                nc.vector.tensor_sub(out=tvar[:, :], in0=tvar[:, :], in1=t1[:, 0:nsegp]).then_inc(VE1, 1)
                if u >= 1:
                    ve_share(u - 1)
                ve.wait_ge(AC1, u + 1)
                nc.vector.reciprocal(out=trstd[:, :], in_=tstd[:, :])
                nc.vector.tensor_scalar_mul(out=At[u % 2][:, :], in0=trstd[:, :], scalar1=w_t[:, fh:fh + 1])
                nc.vector.tensor_scalar_mul(out=t1[:, 0:nsegp], in0=trstd[:, :], scalar1=nw_t[:, fh:fh + 1])
                nc.vector.tensor_mul(out=t1[:, 0:nsegp], in0=tmean[:, :], in1=t1[:, 0:nsegp])
                nc.vector.tensor_scalar_add(out=Ct[u % 2][:, :], in0=t1[:, 0:nsegp], scalar1=b_t[:, fh:fh + 1]).then_inc(VEs, 1)

            ve_share(NUNITS - 1)

        @block.scalar
        def _(ac):
            def do_sqrt(u):
                ac.wait_ge(VE1, u + 1)
                nc.scalar.activation(out=tstd[:, :], in_=tvar[:, :],
                                     func=mybir.ActivationFunctionType.Sqrt,
                                     bias=eps_t[:, 0:1], scale=1.0).then_inc(AC1, 1)
            do_sqrt(0)
            sqrt_at = max(1, int(split * 0.7))
            for u in range(NUNITS):
                ac.wait_ge(VEs, u + 1)
                if u >= 2:
                    ac.wait_ge(ST, 16 * (u - 1))
                xu = xtile[u % 3]
                for k in range(0, split):
                    if k == sqrt_at and u + 1 < NUNITS:
                        do_sqrt(u + 1)
                    s0, ln = segs[k]
                    nc.scalar.activation(
                        out=ytile[u % 2][:, s0:s0 + ln], in_=xu[:, s0:s0 + ln],
                        func=mybir.ActivationFunctionType.Identity,
                        scale=At[u % 2][:, k:k + 1], bias=Ct[u % 2][:, k:k + 1])
                if split <= sqrt_at and u + 1 < NUNITS:
                    do_sqrt(u + 1)
                nc.scalar.activation(out=eps_t[:, :], in_=eps_t[:, :],
                                     func=mybir.ActivationFunctionType.Copy).then_inc(ACs, 1)

    return nc


def kernel(x, affine_weight, affine_bias, change_points):
    x = np.asarray(x, dtype=np.float32)
    w = np.asarray(affine_weight, dtype=np.float32)
    bb = np.asarray(affine_bias, dtype=np.float32)
    cp = np.asarray(change_points)

    segs, cols, extras, nseg, ncol, nsegp, ncolp, ce, co, invl = _plan(cp)
    sig = tuple(s for s, _ in segs)
    if sig not in _cache:
        _cache[sig] = _build(segs, cols, extras, nseg, ncol, nsegp, ncolp)
    nc = _cache[sig]

    wbarr = np.zeros((128, 6), np.float32)
    wbarr[:, 0] = w[0:128]
    wbarr[:, 1] = w[128:256]
    wbarr[:, 2] = -w[0:128]
    wbarr[:, 3] = -w[128:256]
    wbarr[:, 4] = bb[0:128]
    wbarr[:, 5] = bb[128:256]

    in_maps = []
    for i in range(NCORES):
        xt = np.ascontiguousarray(x[i * BPC:(i + 1) * BPC].transpose(0, 2, 1))
        in_maps.append({"xt": xt, "ce": ce, "co": co, "invl": invl, "wb": wbarr})

    res = run_bass_kernel_spmd(nc, in_maps, core_ids=list(range(NCORES)), trace=False)

    y = np.empty((B, S, F), np.float32)
    for i in range(NCORES):
        y[i * BPC:(i + 1) * BPC] = res.results[i]["yt"].transpose(0, 2, 1)
    return y



# revision 28
# speedup vs baseline: 1.7118x; 1.7118x over previous
"""Segment-normalize kernel for trn2, 8 NeuronCores, batch-parallel.

Layout: host transposes x to [B, F, S'] fp16 with positions PERMUTED so
that short segments (grouped by padded length class) come first, long
segments after. Stats:
  - short segments: grouped 3D tensor_reduce (sum over x, and over an
    ACT-squared x^2 tile) — one DVE instruction covers many segments.
  - long segments: per-segment bn_stats (single pass gives sum+ssq).
Normalize is y = x*A[seg] + C[seg] with per-partition scalars, one
instruction per (segment, unit), split across DVE / ACT / GPSIMD by a
greedy makespan balancer. fp16 I/O halves DMA; stats math stays f32.

The device program is specialized at trace time on the boundary list;
compiled NEFFs are cached per boundary signature.
"""

import numpy as np

import concourse.bass as bass
from concourse import mybir
from concourse.bass_utils import run_bass_kernel_spmd

B, S, F = 16, 8192, 256
NCORES = 8
BPC = B // NCORES           # batches per core
NUNITS = BPC * 2            # (batch, feature-half) units per core
EPS = 1e-5

RD_MAX = 56                 # segments with L <= RD_MAX use grouped-reduce stats
RD_Q = 4                    # pad quantum for reduce length classes
RD_CAP = 512                # max free elems per grouped reduce instruction
BN_CAP = 512                # bn_stats hardware max free size
TINY_MAX = 12               # segments with L <= TINY_MAX keep f32 x (fp16
                            # rounding flips signs when within-segment
                            # variance is at noise scale)

# per-instruction overhead (ns) and per-element rate (ns) used by the
# normalize assignment balancer (from the v2 cost model)
DVE_OVH, DVE_RATE = 60.0, 0.2604     # tensor_scalar fp16 4x
ACT_OVH, ACT_RATE = 185.0, 0.8333    # activation Identity
POOL_OVH, POOL_RATE = 120.0, 1.3889  # gpsimd tensor_scalar (eff 0.6)

_cache: dict = {}


def _plan(change_points):
    cp = np.asarray(change_points)
    ind = (cp.sum(axis=0) > 0)
    ind[0] = False
    bpos = np.flatnonzero(ind)
    starts = np.concatenate([[0], bpos]).astype(np.int64)
    ends = np.concatenate([bpos, [S]]).astype(np.int64)
    lens = ends - starts

    rd = [(int(s), int(l)) for s, l in zip(starts, lens) if l <= RD_MAX]
    bn = [(int(s), int(l)) for s, l in zip(starts, lens) if l > RD_MAX]
    rd.sort(key=lambda t: ((t[1] + RD_Q - 1) // RD_Q * RD_Q, t[0]))

    segs = []      # (orig_start, L, Lp, s0p) in NEW seg order
    classes = []   # (Lp, k0, n, blk_off): grouped-reduce instruction groups
    cur = 0
    i = 0
    while i < len(rd):
        Lp = (rd[i][1] + RD_Q - 1) // RD_Q * RD_Q
        j = i
        while j < len(rd) and (rd[j][1] + RD_Q - 1) // RD_Q * RD_Q == Lp:
            j += 1
        ngrp = max(1, RD_CAP // Lp)
        t = i
        while t < j:
            n = min(ngrp, j - t)
            classes.append((Lp, len(segs), n, cur))
            for q in range(t, t + n):
                segs.append((rd[q][0], rd[q][1], Lp, cur))
                cur += Lp
            t += n
        i = j
    nrd = len(segs)
    rd_len = cur
    tiny_len = sum(seg[2] for seg in segs if seg[1] <= TINY_MAX)
    ntiny = sum(1 for seg in segs if seg[1] <= TINY_MAX)

    # bn segments: contiguous, slots padded to even length (gpsimd writes
    # fp16 at 4-byte word granularity; odd-aligned adjacent ops corrupt)
    bn_cols = []   # (s0p, colL) ; col j < nbn is seg (nrd+j)'s first piece
    extras = []    # (owner_global_k, extra_col_idx)
    for (s, l) in bn:
        lp = (l + 1) // 2 * 2
        segs.append((s, l, lp, cur))
        bn_cols.append((cur, min(l, BN_CAP)))
        cur += lp
    nbn = len(bn)
    for j, (s, l) in enumerate(bn):
        off = BN_CAP
        base = segs[nrd + j][3]
        while off < l:
            extras.append((nrd + j, len(bn_cols)))
            bn_cols.append((base + off, min(BN_CAP, l - off)))
            off += BN_CAP
    ncolbn = len(bn_cols)
    Sp = cur
    nseg = nrd + nbn
    nsegp = (nseg + 3) // 4 * 4
    W = nsegp + len(extras)          # tsum/tssq width (extras appended)
    ncolbnp = max(4, (ncolbn + 3) // 4 * 4)

    # host-side constant arrays
    invl = np.ones(W, np.float32)
    for k, (_, L, _, _) in enumerate(segs):
        invl[k] = 1.0 / L
    ce = np.zeros(ncolbnp, np.float32)
    co = np.zeros(ncolbnp, np.float32)
    for j, (s0p, cl) in enumerate(bn_cols):
        ce[j] = (cl + 1) // 2
        co[j] = cl // 2

    # position permutation: perm[slot] = orig position (-1 = zero pad)
    perm = np.full(Sp, -1, np.int64)
    inv_idx = np.zeros(S, np.int64)
    for (s, L, Lp, s0p) in segs:
        perm[s0p:s0p + L] = np.arange(s, s + L)
        inv_idx[s:s + L] = np.arange(s0p, s0p + L)

    # normalize engine assignment: per unit. DVE is stats-bound until the
    # last unit's tail, so units 0..NUNITS-2 split between ACT and Pool
    # (balanced globally); the last unit is balanced across all three
    # engines starting from zero (they all go idle at the end together).
    order = sorted(range(nseg), key=lambda k: -segs[k][1])

    def seg_cost(e, L):
        if e == "d":
            return DVE_OVH + DVE_RATE * L
        if e == "a":
            return ACT_OVH + ACT_RATE * L
        return POOL_OVH + POOL_RATE * L

    assign = []
    load = {"a": 400.0 + 0.8333 * rd_len, "p": 100.0}
    for u in range(NUNITS - 1):
        au = [None] * nseg
        for k in order:
            L = segs[k][1]
            e = min(("a", "p"), key=lambda x: load[x] + seg_cost(x, L))
            au[k] = e
            load[e] += seg_cost(e, L)
        assign.append(au)
    loadl = {"d": 0.0, "a": 0.0, "p": 0.0}
    au = [None] * nseg
    for k in order:
        L = segs[k][1]
        e = min(("d", "a", "p"), key=lambda x: loadl[x] + seg_cost(x, L))
        au[k] = e
        loadl[e] += seg_cost(e, L)
    assign.append(au)
    load["last"] = max(loadl.values())

    return dict(segs=segs, classes=classes, bn_cols=bn_cols, extras=extras,
                nrd=nrd, nbn=nbn, nseg=nseg, nsegp=nsegp, W=W,
                ncolbn=ncolbn, ncolbnp=ncolbnp, rd_len=rd_len, Sp=Sp,
                tiny_len=tiny_len, ntiny=ntiny,
                invl=invl, ce=ce, co=co, perm=perm, inv_idx=inv_idx,
                assign=assign, load=load)


def _build(p):
    f32 = mybir.dt.float32
    f16 = mybir.dt.float16
    segs, classes, bn_cols, extras = p["segs"], p["classes"], p["bn_cols"], p["extras"]
    nrd, nbn, nseg, nsegp, W = p["nrd"], p["nbn"], p["nseg"], p["nsegp"], p["W"]
    ncolbn, ncolbnp, rd_len, Sp = p["ncolbn"], p["ncolbnp"], p["rd_len"], p["Sp"]
    tiny_len, ntiny = p["tiny_len"], p["ntiny"]
    assign = p["assign"]
    Wp = (W + 3) // 4 * 4
    rdl = max(4, (rd_len + 3) // 4 * 4)
    tl = max(4, (tiny_len + 3) // 4 * 4)

    nc = bass.Bass()
    xt = nc.declare_dram_parameter("xt", [BPC, F, Sp], f16, isOutput=False)
    x32_d = nc.declare_dram_parameter("x32", [BPC, F, tl], f32, isOutput=False)
    invl_d = nc.declare_dram_parameter("invl", [Wp], f32, isOutput=False)
    ce_d = nc.declare_dram_parameter("ce", [ncolbnp], f32, isOutput=False)
    co_d = nc.declare_dram_parameter("co", [ncolbnp], f32, isOutput=False)
    wb = nc.declare_dram_parameter("wb", [128, 6], f32, isOutput=False)
    yt = nc.declare_dram_parameter("yt", [BPC, F, Sp], f16, isOutput=True)

    segs_d = [[k for k in range(nseg) if assign[u][k] == "d"] for u in range(NUNITS)]
    segs_a = [[k for k in range(nseg) if assign[u][k] == "a"] for u in range(NUNITS)]
    segs_p = [[k for k in range(nseg) if assign[u][k] == "p"] for u in range(NUNITS)]
    half = max((Sp // 2 + 3) // 4 * 4, (rd_len + 3) // 4 * 4)
    half = min(half, Sp)

    from contextlib import ExitStack
    ctx = ExitStack()
    with ctx:
        xs = [ctx.enter_context(nc.sbuf_tensor(f"xs{i}", [128, Sp], f16)) for i in range(NUNITS)]
        x32s = [ctx.enter_context(nc.sbuf_tensor(f"x32s{i}", [128, tl], f32)) for i in range(NUNITS)]
        x232 = [ctx.enter_context(nc.sbuf_tensor(f"x232_{i}", [128, tl], f32)) for i in range(2)]
        ys = [ctx.enter_context(nc.sbuf_tensor(f"ys{i}", [128, Sp], f16)) for i in range(2)]
        x2s = [ctx.enter_context(nc.sbuf_tensor(f"x2s{i}", [128, rdl], f16)) for i in range(2)]
        s6 = [ctx.enter_context(nc.sbuf_tensor(f"s6_{i}", [128, ncolbnp, 6], f32)) for i in range(2)]
        tsums = [ctx.enter_context(nc.sbuf_tensor(f"tsum{i}", [128, Wp], f32)) for i in range(2)]
        tssqs = [ctx.enter_context(nc.sbuf_tensor(f"tssq{i}", [128, Wp], f32)) for i in range(2)]
        tmeans = [ctx.enter_context(nc.sbuf_tensor(f"tmean{i}", [128, nsegp], f32)) for i in range(2)]
        tvars = [ctx.enter_context(nc.sbuf_tensor(f"tvar{i}", [128, nsegp], f32)) for i in range(2)]
        tstds = [ctx.enter_context(nc.sbuf_tensor(f"tstd{i}", [128, nsegp], f32)) for i in range(2)]
        trstds = [ctx.enter_context(nc.sbuf_tensor(f"trstd{i}", [128, nsegp], f32)) for i in range(2)]
        t1s = [ctx.enter_context(nc.sbuf_tensor(f"t1_{i}", [128, max(nsegp, ncolbnp)], f32)) for i in range(2)]
        t2s = [ctx.enter_context(nc.sbuf_tensor(f"t2_{i}", [128, ncolbnp], f32)) for i in range(2)]
        At = [ctx.enter_context(nc.sbuf_tensor(f"At{i}", [128, nsegp], f32)) for i in range(2)]
        Ct = [ctx.enter_context(nc.sbuf_tensor(f"Ct{i}", [128, nsegp], f32)) for i in range(2)]
        invl_t = ctx.enter_context(nc.sbuf_tensor([128, Wp], f32))
        ce_t = ctx.enter_context(nc.sbuf_tensor([128, ncolbnp], f32))
        co_t = ctx.enter_context(nc.sbuf_tensor([128, ncolbnp], f32))
        w_t = ctx.enter_context(nc.sbuf_tensor([128, 2], f32))
        nw_t = ctx.enter_context(nc.sbuf_tensor([128, 2], f32))
        b_t = ctx.enter_context(nc.sbuf_tensor([128, 2], f32))
        eps_t = ctx.enter_context(nc.sbuf_tensor([128, 1], f32))

        LD = ctx.enter_context(nc.semaphore("LD"))    # +16 per DMA (consts+x)
        X2 = ctx.enter_context(nc.semaphore("X2"))    # ACT squared unit u
        VE1 = ctx.enter_context(nc.semaphore("VE1"))  # DVE stats pre-math done
        AC1 = ctx.enter_context(nc.semaphore("AC1"))  # ACT sqrt done
        VEs = ctx.enter_context(nc.semaphore("VEs"))  # DVE A/C ready
        ND = ctx.enter_context(nc.semaphore("ND"))    # DVE normalize done
        NA = ctx.enter_context(nc.semaphore("NA"))    # ACT normalize done
        NP = ctx.enter_context(nc.semaphore("NP"))    # Pool normalize done
        ST = ctx.enter_context(nc.semaphore("ST"))    # +16 per y store
        block = ctx.enter_context(nc.Block())

        NLDU = 2 if tiny_len else 1
        # LD thresholds (x16 units per DMA): unit-0 A-half (incl x32),
        # unit-0 B-half, consts, then per-unit loads
        HAS_B = half < Sp
        LD_XA0 = 16 * (1 + (1 if tiny_len else 0))
        LD_XB0 = LD_XA0 + (16 if HAS_B else 0)
        LD_CONST = LD_XB0 + 16 * 6

        def ld_unit(u):
            return LD_CONST + 16 * NLDU * u if u >= 1 else LD_XB0

        def bcast(dram, n):
            ap = dram[:]
            return bass.AP(tensor=ap.tensor, offset=ap.offset, ap=[[0, 128], [1, n]])

        def fh_of(u):
            return u % 2

        def unit_dram(u):
            bi, fh = u // 2, u % 2
            return (xt[bi, fh * 128:(fh + 1) * 128, :],
                    yt[bi, fh * 128:(fh + 1) * 128, :],
                    x32_d[bi, fh * 128:(fh + 1) * 128, :])

        @block.sync
        def _(sp):
            xd0, _, x32d0 = unit_dram(0)
            if tiny_len:
                sp.dma_start(out=x32s[0][:, :], in_=x32d0).then_inc(LD, 16)
            sp.dma_start(out=xs[0][:, 0:half], in_=xd0[:, 0:half]).then_inc(LD, 16)
            if HAS_B:
                sp.dma_start(out=xs[0][:, half:Sp], in_=xd0[:, half:Sp]).then_inc(LD, 16)
            sp.dma_start(out=invl_t[:, :], in_=bcast(invl_d, Wp)).then_inc(LD, 16)
            sp.dma_start(out=ce_t[:, :], in_=bcast(ce_d, ncolbnp)).then_inc(LD, 16)
            sp.dma_start(out=co_t[:, :], in_=bcast(co_d, ncolbnp)).then_inc(LD, 16)
            sp.dma_start(out=w_t[:, :], in_=wb[:, 0:2]).then_inc(LD, 16)
            sp.dma_start(out=nw_t[:, :], in_=wb[:, 2:4]).then_inc(LD, 16)
            sp.dma_start(out=b_t[:, :], in_=wb[:, 4:6]).then_inc(LD, 16)
            for u in range(1, NUNITS):
                xd, _, x32d = unit_dram(u)
                sp.dma_start(out=xs[u][:, :], in_=xd).then_inc(LD, 16)
                if tiny_len:
                    sp.dma_start(out=x32s[u][:, :], in_=x32d).then_inc(LD, 16)
            for u in range(NUNITS):
                sp.wait_ge(ND, u + 1)
                sp.wait_ge(NA, u + 1)
                sp.wait_ge(NP, u + 1)
                _, yd, _ = unit_dram(u)
                sp.dma_start(out=yd, in_=ys[u % 2][:, :]).then_inc(ST, 16)

        def norm_segs(engine, seglist, u, ytile):
            Au, Cu = At[u % 2], Ct[u % 2]
            for k in seglist:
                _, L, Lp, s0p = segs[k]
                xu = x32s[u] if k < ntiny else xs[u]
                if engine == "d":
                    nc.vector.tensor_scalar(
                        out=ytile[:, s0p:s0p + L], in0=xu[:, s0p:s0p + L],
                        scalar1=Au[:, k:k + 1], scalar2=Cu[:, k:k + 1],
                        op0=mybir.AluOpType.mult, op1=mybir.AluOpType.add)
                elif engine == "a":
                    nc.scalar.activation(
                        out=ytile[:, s0p:s0p + L], in_=xu[:, s0p:s0p + L],
                        func=mybir.ActivationFunctionType.Identity,
                        scale=Au[:, k:k + 1], bias=Cu[:, k:k + 1])
                else:
                    nc.gpsimd.tensor_scalar(
                        out=ytile[:, s0p:s0p + L], in0=xu[:, s0p:s0p + L],
                        scalar1=Au[:, k:k + 1], scalar2=Cu[:, k:k + 1],
                        op0=mybir.AluOpType.mult, op1=mybir.AluOpType.add)

        @block.vector
        def _(ve):
            nc.vector.memset(eps_t[:, :], EPS)

            def grouped_reduce(dst, src16, src32, u):
                for (Lp, k0, n, blk) in classes:
                    src_t = src32 if blk < tiny_len else src16
                    ap = src_t[:, :]
                    in3 = bass.AP(tensor=ap.tensor, offset=ap.offset + blk,
                                  ap=[list(ap.ap[0]), [Lp, n], [1, Lp]])
                    nc.vector.tensor_reduce(
                        out=dst[:, k0:k0 + n], in_=in3,
                        axis=mybir.AxisListType.X, op=mybir.AluOpType.add)

            def dve_norm(u):
                if u >= 2:
                    ve.wait_ge(ST, 16 * (u - 1))
                norm_segs("d", segs_d[u], u, ys[u % 2])
                nc.vector.memset(t1s[u % 2][:, 0:1], 0.0).then_inc(ND, 1)

            for u in range(NUNITS):
                fh = fh_of(u)
                xu = xs[u]
                tsum, tssq = tsums[u % 2], tssqs[u % 2]
                tmean, tvar = tmeans[u % 2], tvars[u % 2]
                tstd, trstd = tstds[u % 2], trstds[u % 2]
                t1, t2 = t1s[u % 2], t2s[u % 2]
                ve.wait_ge(LD, LD_XA0 if u == 0 else ld_unit(u))
                if u >= 2:
                    ve.wait_ge(NA, u - 1)   # s6/A/C buffer reuse readiness
                    ve.wait_ge(NP, u - 1)
                    ve.wait_ge(ND, u - 1)
                # sums of x over reduce classes (direct into tsum)
                grouped_reduce(tsum, xu, x32s[u], u)
                # bn_stats per long-segment piece
                if u == 0:
                    ve.wait_ge(LD, LD_XB0)
                s = s6[u % 2]
                for j, (s0p, cl) in enumerate(bn_cols):
                    nc.vector.bn_stats(out=s[:, j, :], in_=xu[:, s0p:s0p + cl])
                # sums of x^2 over reduce classes (needs ACT square)
                ve.wait_ge(X2, u + 1)
                grouped_reduce(tssq, x2s[u % 2], x232[u % 2], u)
                if u == 0:
                    ve.wait_ge(LD, LD_CONST)
                if ncolbn:
                    m_e = s[:, :, 1]
                    s_e = s[:, :, 2]
                    m_o = s[:, :, 4]
                    s_o = s[:, :, 5]
                    nc.vector.tensor_mul(out=t1[:, 0:ncolbnp], in0=m_e, in1=ce_t[:, :])
                    nc.vector.tensor_mul(out=t2[:, :], in0=m_o, in1=co_t[:, :])
                    nc.vector.tensor_add(out=tsum[:, nrd:nrd + ncolbn],
                                         in0=t1[:, 0:ncolbn], in1=t2[:, 0:ncolbn])
                    nc.vector.tensor_mul(out=t1[:, 0:ncolbnp], in0=m_e, in1=t1[:, 0:ncolbnp])
                    nc.vector.tensor_mul(out=t2[:, :], in0=m_o, in1=t2[:, :])
                    nc.vector.tensor_add(out=t1[:, 0:ncolbnp], in0=t1[:, 0:ncolbnp], in1=t2[:, :])
                    nc.vector.tensor_add(out=t1[:, 0:ncolbnp], in0=t1[:, 0:ncolbnp], in1=s_e)
                    nc.vector.tensor_add(out=tssq[:, nrd:nrd + ncolbn],
                                         in0=t1[:, 0:ncolbn], in1=s_o)
                for k, j in extras:
                    nc.vector.tensor_add(out=tsum[:, k:k + 1], in0=tsum[:, k:k + 1],
                                         in1=tsum[:, nrd + j:nrd + j + 1])
                    nc.vector.tensor_add(out=tssq[:, k:k + 1], in0=tssq[:, k:k + 1],
                                         in1=tssq[:, nrd + j:nrd + j + 1])
                nc.vector.tensor_mul(out=tmean[:, :], in0=tsum[:, 0:nsegp], in1=invl_t[:, 0:nsegp])
                nc.vector.tensor_mul(out=tvar[:, :], in0=tssq[:, 0:nsegp], in1=invl_t[:, 0:nsegp])
                nc.vector.tensor_mul(out=t1[:, 0:nsegp], in0=tmean[:, :], in1=tmean[:, :])
                nc.vector.tensor_sub(out=tvar[:, :], in0=tvar[:, :], in1=t1[:, 0:nsegp])
                nc.vector.tensor_scalar_max(out=tvar[:, :], in0=tvar[:, :], scalar1=0.0).then_inc(VE1, 1)
                if u >= 1:
                    dve_norm(u - 1)
                ve.wait_ge(AC1, u + 1)
                nc.vector.reciprocal(out=trstd[:, :], in_=tstd[:, :])
                nc.vector.tensor_scalar_mul(out=At[u % 2][:, :], in0=trstd[:, :], scalar1=w_t[:, fh:fh + 1])
                nc.vector.tensor_scalar_mul(out=t1[:, 0:nsegp], in0=trstd[:, :], scalar1=nw_t[:, fh:fh + 1])
                nc.vector.tensor_mul(out=t1[:, 0:nsegp], in0=tmean[:, :], in1=t1[:, 0:nsegp])
                nc.vector.tensor_scalar_add(out=Ct[u % 2][:, :], in0=t1[:, 0:nsegp], scalar1=b_t[:, fh:fh + 1]).then_inc(VEs, 1)

            dve_norm(NUNITS - 1)

        @block.scalar
        def _(ac):
            def square(u):
                ac.wait_ge(LD, LD_XA0 if u == 0 else ld_unit(u))
                if u >= 2:
                    ac.wait_ge(VE1, u - 1)  # x2 buffer consumed
                last = None
                if tiny_len:
                    last = nc.scalar.activation(
                        out=x232[u % 2][:, 0:tiny_len], in_=x32s[u][:, 0:tiny_len],
                        func=mybir.ActivationFunctionType.Square)
                if rd_len > tiny_len:
                    last = nc.scalar.activation(
                        out=x2s[u % 2][:, tiny_len:rd_len], in_=xs[u][:, tiny_len:rd_len],
                        func=mybir.ActivationFunctionType.Square)
                if last is None:
                    last = nc.scalar.activation(out=eps_t[:, :], in_=eps_t[:, :],
                                                func=mybir.ActivationFunctionType.Copy)
                last.then_inc(X2, 1)

            square(0)
            for u in range(NUNITS):
                if u + 1 < NUNITS:
                    square(u + 1)
                ac.wait_ge(VE1, u + 1)
                nc.scalar.activation(out=tstds[u % 2][:, :], in_=tvars[u % 2][:, :],
                                     func=mybir.ActivationFunctionType.Sqrt,
                                     bias=eps_t[:, 0:1], scale=1.0).then_inc(AC1, 1)
                ac.wait_ge(VEs, u + 1)
                if u >= 2:
                    ac.wait_ge(ST, 16 * (u - 1))
                norm_segs("a", segs_a[u], u, ys[u % 2])
                nc.scalar.activation(out=eps_t[:, :], in_=eps_t[:, :],
                                     func=mybir.ActivationFunctionType.Copy).then_inc(NA, 1)

        @block.gpsimd
        def _(g):
            for u in range(NUNITS):
                g.wait_ge(VEs, u + 1)
                if u >= 2:
                    g.wait_ge(ST, 16 * (u - 1))
                norm_segs("p", segs_p[u], u, ys[u % 2])
                g.drain().then_inc(NP, 1)

    return nc


def _prep_core(x_core, p):
    """x_core [BPC, S, F] f32 -> permuted padded fp16 [BPC, F, Sp] plus the
    f32 tiny-region prefix [BPC, F, tl]."""
    Sp, perm, tiny_len = p["Sp"], p["perm"], p["tiny_len"]
    tl = max(4, (tiny_len + 3) // 4 * 4)
    src = np.ascontiguousarray(x_core.transpose(0, 2, 1))
    xt = np.zeros((BPC, F, Sp), np.float16)
    valid = perm >= 0
    xt[:, :, valid] = src[:, :, perm[valid]].astype(np.float16)
    x32 = np.zeros((BPC, F, tl), np.float32)
    tv = valid[:tiny_len]
    x32[:, :, :tiny_len][:, :, tv] = src[:, :, perm[:tiny_len][tv]]
    return xt, x32


def kernel(x, affine_weight, affine_bias, change_points):
    x = np.asarray(x, dtype=np.float32)
    w = np.asarray(affine_weight, dtype=np.float32)
    bb = np.asarray(affine_bias, dtype=np.float32)
    cp = np.asarray(change_points)

    p = _plan(cp)
    sig = tuple(s for s, _, _, _ in p["segs"]) + (p["nrd"],)
    if sig not in _cache:
        _cache[sig] = _build(p)
    nc = _cache[sig]

    wbarr = np.zeros((128, 6), np.float32)
    wbarr[:, 0] = w[0:128]
    wbarr[:, 1] = w[128:256]
    wbarr[:, 2] = -w[0:128]
    wbarr[:, 3] = -w[128:256]
    wbarr[:, 4] = bb[0:128]
    wbarr[:, 5] = bb[128:256]

    Wp = (p["W"] + 3) // 4 * 4
    invl = np.ones(Wp, np.float32)
    invl[:p["W"]] = p["invl"]

    in_maps = []
    for i in range(NCORES):
        xt, x32 = _prep_core(x[i * BPC:(i + 1) * BPC], p)
        in_maps.append({"xt": xt, "x32": x32, "invl": invl, "ce": p["ce"],
                        "co": p["co"], "wb": wbarr})

    res = run_bass_kernel_spmd(nc, in_maps, core_ids=list(range(NCORES)), trace=False)

    inv_idx = p["inv_idx"]
    y = np.empty((B, S, F), np.float32)
    for i in range(NCORES):
        yp = res.results[i]["yt"]                      # [BPC, F, Sp] f16
        y[i * BPC:(i + 1) * BPC] = yp[:, :, inv_idx].transpose(0, 2, 1)
    return y
